# revision 6
# baseline (speedup 1.0000x reference)
"""PolylineEncoder TRN2 kernel.

Reference: 3x [Linear(256->128)+ReLU -> mask invalid to -inf -> max-pool over
20 nodes -> concat(h, pooled_bcast) -> zero invalid], final masked max-pool,
zero all-invalid polylines.  Output [16,1024,256] f32.

Strategy:
- Host: replace every invalid node's features with the first valid node's
  features.  Then every column is a copy of a valid column through all layers
  (column-local compute + shared pooled term), so the unmasked max over all 20
  nodes equals the reference's masked max (ReLU>=0, duplicates don't change a
  max).  No masking on device at all.  All-invalid polylines are zeroed on host.
- Output = concat(pooled3, pooled3): layer-3 concat/zero never affects the
  result, so the device only computes pooled3 [128, npoly]; host duplicates.
- Feature-major layout everywhere (features on partitions, tokens on free dim):
  no on-chip transposes.  Host pre-transposes x per core; the tiny [128,2048]
  per-core result is transposed back on host.
- Data parallel over scenes: 2 scenes per core x 8 cores.
- f32r matmuls (1 cycle/row when N>=256), K=256 split into 2 accumulating
  matmuls.  Pooled-broadcast term feeds the second matmul of the next layer
  via a stride-0 AP (no materialization).
"""

import numpy as np

import concourse.bacc as bacc
import concourse.mybir as mybir
import concourse.tile as tile
from concourse import bass_utils

N_SC, N_MP, N_NODE, HIDDEN = 16, 1024, 20, 256
HALF = HIDDEN // 2
N_CORES = 8
SC_PER_CORE = N_SC // N_CORES            # 2
NPOLY = SC_PER_CORE * N_MP               # 2048 polylines per core
NTOK = NPOLY * N_NODE                    # 40960 tokens per core
TILE_POLY = 25                           # 25 polylines = 500 tokens <= 512 psum
FULL_TILES = NPOLY // TILE_POLY          # 81
REM_POLY = NPOLY - FULL_TILES * TILE_POLY  # 23

_FP32 = mybir.dt.float32
_F32R = mybir.dt.float32r


def _emit(nc, xT, wcat, bias, out):
    relu = mybir.ActivationFunctionType.Relu
    ax_x = mybir.AxisListType.X
    op_max = mybir.AluOpType.max

    with tile.TileContext(nc) as tc:
        with tc.tile_pool(name="const", bufs=1) as kpool, \
             tc.tile_pool(name="sb", bufs=3) as sb, \
             tc.tile_pool(name="ps", bufs=2, space="PSUM") as pp, \
             tc.tile_pool(name="stage", bufs=1) as stage:

            wr = kpool.tile([128, 6 * HALF], _F32R)
            nc.sync.dma_start(wr, wcat.bitcast(_F32R))
            bt = kpool.tile([128, 3], _FP32)
            nc.sync.dma_start(bt, bias)
            outS = stage.tile([128, NPOLY], _FP32)
            xTr = xT.bitcast(_F32R)

            for t in range(FULL_TILES + 1):
                npoly = TILE_POLY if t < FULL_TILES else REM_POLY
                ntok = npoly * N_NODE
                c0 = t * TILE_POLY * N_NODE

                xA = sb.tile([128, ntok], _F32R, tag="xA")
                nc.sync.dma_start(xA, xTr[0:128, c0:c0 + ntok])
                xB = sb.tile([128, ntok], _F32R, tag="xB")
                nc.sync.dma_start(xB, xTr[128:256, c0:c0 + ntok])

                # layer 0: h0 = relu(W0 @ x + b0)
                ps0 = pp.tile([128, ntok], _FP32, tag="ps")
                nc.tensor.matmul(ps0, wr[:, 0:128], xA,
                                 start=True, stop=False)
                nc.tensor.matmul(ps0, wr[:, 128:256], xB,
                                 start=False, stop=True)
                h0 = sb.tile([128, ntok], _F32R, tag="h0")
                nc.scalar.activation(h0, ps0, relu, bias=bt[:, 0:1])
                p0 = sb.tile([128, npoly], _F32R, tag="p0")
                nc.vector.tensor_reduce(
                    p0, h0.rearrange("p (g n) -> p g n", n=N_NODE),
                    axis=ax_x, op=op_max)

                # layer 1: h1 = relu(W1 @ concat(h0, bcast(p0)) + b1)
                ps1 = pp.tile([128, ntok], _FP32, tag="ps")
                nc.tensor.matmul(ps1, wr[:, 256:384], h0,
                                 start=True, stop=False)
                p0b = p0.unsqueeze(2).to_broadcast((128, npoly, N_NODE))
                nc.tensor.matmul(ps1, wr[:, 384:512], p0b,
                                 start=False, stop=True)
                h1 = sb.tile([128, ntok], _F32R, tag="h1")
                nc.scalar.activation(h1, ps1, relu, bias=bt[:, 1:2])
                p1 = sb.tile([128, npoly], _F32R, tag="p1")
                nc.vector.tensor_reduce(
                    p1, h1.rearrange("p (g n) -> p g n", n=N_NODE),
                    axis=ax_x, op=op_max)

                # layer 2: pooled3 = maxpool(relu(W2 @ concat(h1, bcast(p1)) + b2))
                ps2 = pp.tile([128, ntok], _FP32, tag="ps")
                nc.tensor.matmul(ps2, wr[:, 512:640], h1,
                                 start=True, stop=False)
                p1b = p1.unsqueeze(2).to_broadcast((128, npoly, N_NODE))
                nc.tensor.matmul(ps2, wr[:, 640:768], p1b,
                                 start=False, stop=True)
                h2 = sb.tile([128, ntok], _FP32, tag="h2")
                nc.scalar.activation(h2, ps2, relu, bias=bt[:, 2:3])
                nc.vector.tensor_reduce(
                    outS[:, t * TILE_POLY:t * TILE_POLY + npoly],
                    h2.rearrange("p (g n) -> p g n", n=N_NODE),
                    axis=ax_x, op=op_max)

            nc.sync.dma_start(out, outS)


_NC_CACHE = None


def _build_nc():
    global _NC_CACHE
    if _NC_CACHE is not None:
        return _NC_CACHE
    nc = bacc.Bacc("TRN2", target_bir_lowering=False, debug=False)
    xT = nc.dram_tensor("xT", (HIDDEN, NTOK), _FP32, kind="ExternalInput").ap()
    wcat = nc.dram_tensor("wcat", (128, 6 * HALF), _FP32,
                          kind="ExternalInput").ap()
    bias = nc.dram_tensor("bias", (128, 3), _FP32, kind="ExternalInput").ap()
    out = nc.dram_tensor("out", (128, NPOLY), _FP32,
                         kind="ExternalOutput").ap()
    _emit(nc, xT, wcat, bias, out)
    nc.finalize()
    _NC_CACHE = nc
    return nc


def kernel(**inputs) -> np.ndarray:
    x = np.asarray(inputs["x"], dtype=np.float32)
    invalid = np.asarray(inputs["invalid"]).astype(bool)
    W0 = np.asarray(inputs["W0"], dtype=np.float32)
    b0 = np.asarray(inputs["b0"], dtype=np.float32)
    W1 = np.asarray(inputs["W1"], dtype=np.float32)
    b1 = np.asarray(inputs["b1"], dtype=np.float32)
    W2 = np.asarray(inputs["W2"], dtype=np.float32)
    b2 = np.asarray(inputs["b2"], dtype=np.float32)

    # replace invalid nodes with the first valid node's features
    valid = ~invalid
    n0 = np.argmax(valid, axis=-1)                       # [16,1024]
    sub = np.take_along_axis(x, n0[..., None, None], axis=2)
    xf = np.where(invalid[..., None], sub, x)            # [16,1024,20,256]

    # lhsT blocks: out = lhsT.T @ rhs, so lhsT[k,m] = W[m,k]
    wcat = np.ascontiguousarray(np.concatenate(
        [W0[:, :HALF].T, W0[:, HALF:].T,
         W1[:, :HALF].T, W1[:, HALF:].T,
         W2[:, :HALF].T, W2[:, HALF:].T], axis=1))       # [128, 768]
    bias = np.ascontiguousarray(np.stack([b0, b1, b2], axis=1))  # [128, 3]

    in_maps = []
    for c in range(N_CORES):
        xc = xf[SC_PER_CORE * c:SC_PER_CORE * (c + 1)].reshape(NTOK, HIDDEN)
        in_maps.append({
            "xT": np.ascontiguousarray(xc.T),            # [256, 40960]
            "wcat": wcat,
            "bias": bias,
        })

    nc = _build_nc()
    res = bass_utils.run_bass_kernel_spmd(nc, in_maps,
                                          core_ids=list(range(N_CORES)))

    pooled = np.concatenate(
        [np.asarray(res.results[c]["out"]).T.reshape(SC_PER_CORE, N_MP, HALF)
         for c in range(N_CORES)], axis=0)               # [16,1024,128]
    full = np.concatenate([pooled, pooled], axis=-1)     # [16,1024,256]
    full[invalid.all(axis=-1)] = 0.0
    return full.astype(np.float32)
